# revision 1
# baseline (speedup 1.0000x reference)
"""BiMamba block Trainium2 Bass kernel (8 NeuronCores, SPMD).

Sharding: 8 cores = 2 directions x 4 batch elements; each core runs the full
Mamba block for one (direction, batch) pair, feature-major, including its
direction's half of the final fused projection (out_proj and the fused matmul
are merged via a host-precomputed (fus_w_half @ out_w) weight). The backward
cores consume/produce time-flipped data (host flips). Host gather:
out[b] = partial_f[b].T + flip_L(partial_b[b].T); residual x and fus_b are
added on device/host as noted in gather().

Per-core pipeline (L=1024, scan phase processed as 2 halves of 512):
  P0 LN (token-major, ACT accum_out stats) -> PE transpose to feature-major
  P1 in_proj (PE, bf16, both halves; xi tiles evacuated first so conv starts early)
  P2 causal depthwise conv (DVE tensor_scalar taps + adds, sigmoid-gated)
  P3 xproj (PE) -> dt/B/C; B/C rows broadcast across partitions via a DRAM
     round-trip DMA with stride-0 reads
  P4 dtproj (PE) + softplus composed as ln(1+exp(x)) (ACT) + delta*u (GPSIMD)
  P5 selective scan: per (d-tile, state n) DVE tensor_tensor_scan
     (h_t = a_t*h_{t-1} + b_t, fp32 internal state), a = exp(A*delta) on ACT
     with per-partition scale, b = delta*u*B_n on DVE (broadcast APs);
     readout y += C_n * h_n with multiplies on DVE and accumulate-adds
     alternating DVE/GPSIMD into two accumulators; bf16 storage throughout
     with fp32 a-tiles (decay factors must not be bf16-quantized)
  P6 gate (xc*D + y) * silu(z)
  P7 merged output projection (PE) + fus_b bias, fp32 out

Phases are emitted interleaved (p3p4(1) before p7(0)) so half-1 scan prep
overlaps half-0's output projection. All engine assignments were tuned
against the TimelineSim cost model (~1.09 ms/core predicted).
"""

import os
import sys

import numpy as np
import ml_dtypes

for _p in ("/opt/trn_rl_repo", "/root/.axon_site/_ro/trn_rl_repo"):
    if os.path.isdir(_p) and _p not in sys.path:
        sys.path.append(_p)

import concourse.bass as bass
import concourse.mybir as mybir
import concourse.tile as tile
from concourse import bacc
from concourse.masks import make_identity
from concourse import library_config

BF16 = mybir.dt.bfloat16
F32 = mybir.dt.float32
AFT = mybir.ActivationFunctionType
ALU = mybir.AluOpType
NPBF = ml_dtypes.bfloat16

D_MODEL = 1024
D_STATE = 16
D_CONV = 4
D_INNER = 2048
DT_RANK = 64
B_SZ = 4
L = 1024
HALF = 512
LN_EPS = 1e-5
DT = D_INNER // 128          # 16 d-tiles
MT = 2 * D_INNER // 128      # 32 in_proj out tiles
KM = D_MODEL // 128          # 8 k-tiles over d_model
DMT = D_MODEL // 128         # 8 d_model out tiles
NGRP = 2                     # d-tile groups in scan readout
AT_BUFS = 3
BCP_BUFS = 4
GDT = DT // NGRP             # 8 d-tiles per group


def build_bass():
    nc = bacc.Bacc("TRN2", target_bir_lowering=False, debug=False,
                   enable_asserts=False, num_devices=8)

    # ---- DRAM I/O ----
    x_t = nc.dram_tensor("x_t", [L, D_MODEL], BF16, kind="ExternalInput").ap()
    w_in_T = nc.dram_tensor("w_in_T", [D_MODEL, 2 * D_INNER], BF16, kind="ExternalInput").ap()
    cvec = nc.dram_tensor("cvec", [128, MT], F32, kind="ExternalInput").ap()
    convw = nc.dram_tensor("convw", [128, DT * D_CONV], F32, kind="ExternalInput").ap()
    convb = nc.dram_tensor("convb", [128, DT], F32, kind="ExternalInput").ap()
    w_xproj_T = nc.dram_tensor("w_xproj_T", [D_INNER, 96], BF16, kind="ExternalInput").ap()
    w_dt_T = nc.dram_tensor("w_dt_T", [DT_RANK, D_INNER], BF16, kind="ExternalInput").ap()
    dt_b = nc.dram_tensor("dt_b", [128, DT], F32, kind="ExternalInput").ap()
    A_sc = nc.dram_tensor("A_sc", [128, DT * D_STATE], F32, kind="ExternalInput").ap()
    D_sc = nc.dram_tensor("D_sc", [128, DT], F32, kind="ExternalInput").ap()
    w_comb = nc.dram_tensor("w_comb", [D_INNER, D_MODEL], BF16, kind="ExternalInput").ap()
    fus_b = nc.dram_tensor("fus_b", [128, DMT], F32, kind="ExternalInput").ap()
    part_out = nc.dram_tensor("part_out", [D_MODEL, L], F32, kind="ExternalOutput").ap()
    bc_dram = nc.dram_tensor("bc_scratch", [2, 32, HALF], BF16, kind="Internal").ap()

    with tile.TileContext(nc) as tc:
        _build(tc, x_t, w_in_T, cvec, convw, convb, w_xproj_T, w_dt_T, dt_b,
               A_sc, D_sc, w_comb, fus_b, part_out, bc_dram)
    nc.compile()
    return nc


def _build(tc, x_t, w_in_T, cvec, convw, convb, w_xproj_T, w_dt_T, dt_b,
           A_sc, D_sc, w_comb, fus_b, part_out, bc_dram):
    nc = tc.nc

    cp = tc.alloc_tile_pool(name="consts", bufs=1)
    # constants / small weights, resident for the whole kernel
    ident = cp.tile([128, 128], BF16)
    make_identity(nc, ident)
    cvec_sb = cp.tile([128, MT], F32)
    convw_sb = cp.tile([128, DT * D_CONV], F32)
    convb_sb = cp.tile([128, DT], F32)
    dtb_sb = cp.tile([128, DT], F32)
    A_sb = cp.tile([128, DT * D_STATE], F32)
    D_sb = cp.tile([128, DT], F32)
    fusb_sb = cp.tile([128, DMT], F32)
    # xproj weight as 16 partition-tiles: dram (2048, 96) -> sbuf [128, 16*96]
    wxp = cp.tile([128, DT * 96], BF16)
    wdt = cp.tile([DT_RANK, D_INNER], BF16)
    # per-(d-tile, n) scan carry state between halves
    sc_all = cp.tile([128, DT * D_STATE], F32)

    bigG = tc.alloc_tile_pool(name="bigG", bufs=1)
    g = bigG.tile([128, DT * L], BF16)                 # silu(z), resident
    xcp = tc.alloc_tile_pool(name="xcp", bufs=1)
    xc = xcp.tile([128, DT * L], BF16)                 # conv output, resident
    xip = tc.alloc_tile_pool(name="xip", bufs=1)
    xi = xip.tile([128, DT * (L + 3)], BF16)           # conv input w/ halo

    # ---------------- P0: LN + transpose ----------------
    xnTp = tc.alloc_tile_pool(name="xnTp", bufs=1)
    xnT = xnTp.tile([128, KM * L], BF16)               # feature-major LN output
    with tc.tile_pool(name="p0", bufs=2) as p0, \
         tc.tile_pool(name="p0x", bufs=1) as p0x, \
         tc.tile_pool(name="p0s", bufs=4) as p0s, \
         tc.tile_pool(name="psT", bufs=4, space="PSUM") as psT:

        xt = p0x.tile([128, KM * L], BF16)             # whole x, token-major
        for tt in range(8):
            nc.sync.dma_start(xt[:, tt * D_MODEL:(tt + 1) * D_MODEL],
                              x_t[tt * 128:(tt + 1) * 128, :])
        nc.sync.dma_start(cvec_sb[:], cvec)
        nc.sync.dma_start(convw_sb[:], convw)
        nc.sync.dma_start(convb_sb[:], convb)
        nc.sync.dma_start(dtb_sb[:], dt_b)
        nc.sync.dma_start(A_sb[:], A_sc)
        nc.sync.dma_start(D_sb[:], D_sc)
        nc.sync.dma_start(fusb_sb[:], fus_b)
        for k in range(DT):
            nc.sync.dma_start(wxp[:, k * 96:(k + 1) * 96], w_xproj_T[k * 128:(k + 1) * 128, :])
        nc.sync.dma_start(wdt[:], w_dt_T)
        for tt in range(8):
            xts = xt[:, tt * D_MODEL:(tt + 1) * D_MODEL]
            sq = p0.tile([128, D_MODEL], BF16, tag="sq")
            ssum = p0s.tile([128, 1], F32, tag="ssum")
            nc.vector.reduce_sum(ssum[:], xts, axis=mybir.AxisListType.X)
            ssq = p0s.tile([128, 1], F32, tag="ssq")
            nc.scalar.activation(sq[:], xts, AFT.Square, accum_out=ssq[:])
            mu = p0s.tile([128, 1], F32, tag="mu")
            nc.vector.tensor_scalar_mul(mu[:], ssum[:], 1.0 / D_MODEL)
            var = p0s.tile([128, 1], F32, tag="var")
            musq = p0s.tile([128, 1], F32, tag="musq")
            nc.vector.tensor_mul(musq[:], mu[:], mu[:])
            nc.vector.tensor_scalar(var[:], ssq[:], 1.0 / D_MODEL, LN_EPS, ALU.mult, ALU.add)
            nc.vector.tensor_sub(var[:], var[:], musq[:])
            std = p0s.tile([128, 1], F32, tag="std")
            nc.scalar.sqrt(std[:], var[:])
            rstd = p0s.tile([128, 1], F32, tag="rstd")
            nc.vector.reciprocal(rstd[:], std[:])
            nbias = p0s.tile([128, 1], F32, tag="nbias")
            nc.vector.tensor_mul(nbias[:], mu[:], rstd[:])
            nc.vector.tensor_scalar_mul(nbias[:], nbias[:], -1.0)
            xn = p0.tile([128, D_MODEL], BF16, tag="xn")
            nc.scalar.activation(xn[:], xts, AFT.Identity, bias=nbias[:], scale=rstd[:])
            for db in range(KM):
                pt = psT.tile([128, 128], BF16, tag="tr")
                nc.tensor.transpose(pt[:], xn[:, db * 128:(db + 1) * 128], ident[:])
                nc.vector.tensor_copy(xnT[:, db * L + tt * 128:db * L + (tt + 1) * 128], pt[:])

    # ---------------- P1: in_proj (both halves) ----------------
    with tc.tile_pool(name="p1", bufs=2) as p1, \
         tc.tile_pool(name="winp", bufs=8) as winp, \
         tc.tile_pool(name="psA", bufs=4, space="PSUM") as psA:

        win = [winp.tile([128, 2 * D_INNER], BF16, tag="win", name=f"win{k}") for k in range(KM)]
        for k in range(KM):
            nc.sync.dma_start(win[k][:], w_in_T[k * 128:(k + 1) * 128, :])
        # zero conv halo columns for half 0
        for i in range(DT):
            nc.vector.memset(xi[:, i * (L + 3):i * (L + 3) + 3], 0.0)
        for m, h in ([(m, h) for h in range(2) for m in range(DT)]
                     + [(m, h) for h in range(2) for m in range(DT, MT)]):
            ps = psA.tile([128, HALF], F32, tag="mm")
            for k in range(KM):
                nc.tensor.matmul(ps[:], win[k][:, m * 128:(m + 1) * 128],
                                 xnT[:, k * L + h * HALF:k * L + (h + 1) * HALF],
                                 start=(k == 0), stop=(k == KM - 1))
            if m < DT:
                dst = xi[:, m * (L + 3) + 3 + h * HALF: m * (L + 3) + 3 + (h + 1) * HALF]
                nc.scalar.activation(dst, ps[:], AFT.Identity, bias=cvec_sb[:, m:m + 1])
            else:
                z = m - DT
                sgt = p1.tile([128, HALF], BF16, tag="sgt")
                nc.scalar.activation(sgt[:], ps[:], AFT.Sigmoid, bias=cvec_sb[:, m:m + 1])
                zc = p1.tile([128, HALF], BF16, tag="zc")
                nc.scalar.activation(zc[:], ps[:], AFT.Identity, bias=cvec_sb[:, m:m + 1])
                nc.gpsimd.tensor_mul(g[:, z * L + h * HALF:z * L + (h + 1) * HALF],
                                     zc[:], sgt[:])

    xnTp.release()
    # ---------------- P2: depthwise causal conv ----------------
    if True:
        cvp = cp
        for h in range(2):
            for i in range(DT):
                base = i * (L + 3)
                acc = cvp.tile([128, HALF], BF16, tag="acc", bufs=2)
                tmp = cvp.tile([128, HALF], BF16, tag="ctmp", bufs=2)
                o = base + 3 + h * HALF
                nc.vector.tensor_scalar_mul(acc[:], xi[:, o:o + HALF],
                                            convw_sb[:, i * D_CONV + 3:i * D_CONV + 4])
                for kk in range(3):
                    s = 3 - kk
                    nc.vector.tensor_scalar_mul(tmp[:], xi[:, o - s:o + HALF - s],
                                                convw_sb[:, i * D_CONV + kk:i * D_CONV + kk + 1])
                    nc.vector.tensor_add(acc[:], acc[:], tmp[:])
                sgc = cvp.tile([128, HALF], BF16, tag="sgc", bufs=1)
                nc.scalar.activation(sgc[:], acc[:], AFT.Sigmoid, bias=convb_sb[:, i:i + 1])
                nc.vector.scalar_tensor_tensor(
                    xc[:, i * L + h * HALF:i * L + (h + 1) * HALF], acc[:],
                    convb_sb[:, i:i + 1], sgc[:], op0=ALU.add, op1=ALU.mult)
    xip.release()
    hp = tc.alloc_tile_pool(name="hp", bufs=1)
    gp = tc.alloc_tile_pool(name="gp", bufs=1)

    # ---------------- per-half scan pipeline ----------------
    with tc.tile_pool(name="dtp", bufs=1) as dtp, \
         tc.tile_pool(name="bcp", bufs=BCP_BUFS) as bcp, \
         tc.tile_pool(name="scn", bufs=1) as scn, \
         tc.tile_pool(name="ap_", bufs=2) as ap_, \
         tc.tile_pool(name="outp", bufs=4) as outp, \
         tc.tile_pool(name="psB", bufs=4, space="PSUM") as psB, \
         tc.tile_pool(name="psX", bufs=2, space="PSUM") as psX:

        dms = {}

        def p3p4(h):
            # ---- P3: xproj ----
            psx = psX.tile([96, HALF], F32, tag="xp", name=f"psx{h}")
            for k in range(DT):
                nc.tensor.matmul(psx[:], wxp[:, k * 96:(k + 1) * 96],
                                 xc[:, k * L + h * HALF:k * L + (h + 1) * HALF],
                                 start=(k == 0), stop=(k == DT - 1))
            dt_sb = dtp.tile([DT_RANK, HALF], BF16, tag="dt", name=f"dt_sb{h}")
            nc.scalar.copy(dt_sb[:], psx[0:DT_RANK, :])
            bc_sb = dtp.tile([32, HALF], BF16, tag="bc", name=f"bc_sb{h}")
            nc.scalar.copy(bc_sb[:], psx[DT_RANK:96, :])
            nc.sync.dma_start(bc_dram[h], bc_sb[:])

            # ---- P4: dtproj + softplus + delta*u ----
            dmega = hp.tile([128, DT * HALF], BF16, tag="dmega", name=f"dmega{h}")
            dumega = hp.tile([128, DT * HALF], BF16, tag="dumega", name=f"dumega{h}")
            for i in range(DT):
                psd = psB.tile([128, HALF], F32, tag="mmB", name=f"psd{h}_{i}")
                nc.tensor.matmul(psd[:], wdt[:, i * 128:(i + 1) * 128], dt_sb[:],
                                 start=True, stop=True)
                et = ap_.tile([128, HALF], F32, tag="et", bufs=2, name=f"et{h}_{i}")
                nc.scalar.activation(et[:], psd[:], AFT.Exp, bias=dtb_sb[:, i:i + 1])
                nc.vector.tensor_scalar_add(et[:], et[:], 1.0)
                nc.scalar.activation(dmega[:, i * HALF:(i + 1) * HALF], et[:], AFT.Ln)
                nc.gpsimd.tensor_mul(dumega[:, i * HALF:(i + 1) * HALF],
                                     dmega[:, i * HALF:(i + 1) * HALF],
                                     xc[:, i * L + h * HALF:i * L + (h + 1) * HALF])
            dms[h] = (dmega, dumega)

        def scan_gate(h):
            dmega, dumega = dms[h]
            # ---- P5: selective scan ----
            hmega = scn.tile([128, DT * HALF], BF16, tag="hmega", name=f"hmega{h}")
            ymega = scn.tile([128, DT * HALF], BF16, tag="ymega", name=f"ymega{h}")
            ypool = gp.tile([128, DT * HALF], BF16, tag="gated", name=f"ypool{h}")
            for n in range(D_STATE):
                brep = bcp.tile([128, HALF], BF16, tag="brep", name=f"brep{h}_{n}")
                nc.sync.dma_start(brep[:], bc_dram[h, n:n + 1, :].broadcast_to((128, HALF)))
                crep = bcp.tile([128, HALF], BF16, tag="crep", name=f"crep{h}_{n}")
                nc.sync.dma_start(crep[:], bc_dram[h, D_STATE + n:D_STATE + n + 1, :].broadcast_to((128, HALF)))
                for gi in range(NGRP):
                    gsl = slice(gi * GDT * HALF, (gi + 1) * GDT * HALF)
                    bt = scn.tile([128, GDT * HALF], BF16, tag="bt", name=f"bt{h}_{n}_{gi}")
                    nc.vector.tensor_tensor(
                        bt[:].rearrange("p (i t) -> p i t", i=GDT),
                        dumega[:, gsl].rearrange("p (i t) -> p i t", i=GDT),
                        brep[:].unsqueeze(1).broadcast_to((128, GDT, HALF)),
                        op=ALU.mult)
                    for ii in range(GDT):
                        i = gi * GDT + ii
                        at = ap_.tile([128, HALF], F32, tag="at", bufs=AT_BUFS, name=f"at{h}_{n}_{i}")
                        nc.scalar.activation(at[:], dmega[:, i * HALF:(i + 1) * HALF],
                                             AFT.Exp, scale=A_sb[:, i * D_STATE + n:i * D_STATE + n + 1])
                        init = 0.0 if h == 0 else sc_all[:, i * D_STATE + n:i * D_STATE + n + 1]
                        nc.vector.tensor_tensor_scan(
                            hmega[:, i * HALF:(i + 1) * HALF], at[:],
                            bt[:, ii * HALF:(ii + 1) * HALF], init,
                            op0=ALU.mult, op1=ALU.add)
                        if h == 0:
                            nc.gpsimd.tensor_copy(
                                sc_all[:, i * D_STATE + n:i * D_STATE + n + 1],
                                hmega[:, (i + 1) * HALF - 1:(i + 1) * HALF])
                    # readout: C_n * h_n on DVE; accumulate-adds alternate
                    # between DVE (even n -> ymega) and GPSIMD (odd n -> ypool)
                    acc = ymega if n % 2 == 0 else ypool
                    if n < 2:
                        nc.vector.tensor_tensor(
                            acc[:, gsl].rearrange("p (i t) -> p i t", i=GDT),
                            hmega[:, gsl].rearrange("p (i t) -> p i t", i=GDT),
                            crep[:].unsqueeze(1).broadcast_to((128, GDT, HALF)),
                            op=ALU.mult)
                    else:
                        tmpm = scn.tile([128, GDT * HALF], BF16, tag="tm", bufs=2,
                                        name=f"tm{h}_{n}_{gi}")
                        nc.vector.tensor_tensor(
                            tmpm[:].rearrange("p (i t) -> p i t", i=GDT),
                            hmega[:, gsl].rearrange("p (i t) -> p i t", i=GDT),
                            crep[:].unsqueeze(1).broadcast_to((128, GDT, HALF)),
                            op=ALU.mult)
                        eng = nc.vector if n % 2 == 0 else nc.gpsimd
                        eng.tensor_add(acc[:, gsl], acc[:, gsl], tmpm[:])
            nc.vector.tensor_add(ymega[:, 0:GDT * HALF], ymega[:, 0:GDT * HALF],
                                 ypool[:, 0:GDT * HALF])
            nc.vector.tensor_add(ymega[:, GDT * HALF:], ymega[:, GDT * HALF:],
                                 ypool[:, GDT * HALF:])
            # ---- P6: gate ----
            gated = gp.tile([128, DT * HALF], BF16, tag="gated", name=f"gated{h}")
            for i in range(DT):
                isl = slice(i * HALF, (i + 1) * HALF)
                tgt = ap_.tile([128, HALF], BF16, tag="tgt", bufs=1, name=f"tgt{h}_{i}")
                nc.vector.scalar_tensor_tensor(tgt[:], xc[:, i * L + h * HALF:i * L + (h + 1) * HALF],
                                               D_sb[:, i:i + 1], ymega[:, isl],
                                               op0=ALU.mult, op1=ALU.add)
                nc.vector.tensor_mul(gated[:, isl], tgt[:], g[:, i * L + h * HALF:i * L + (h + 1) * HALF])
            dms[h] = (None, None)
            return gated

        def p7(h, gated):
            hs = slice(h * HALF, (h + 1) * HALF)
            for grp in range(2):
                psos = [psB.tile([128, HALF], F32, tag="mmB", name=f"pso{h}_{grp}_{j}")
                        for j in range(4)]
                for k in range(DT):
                    wok = outp.tile([128, 512], BF16, tag="wo", bufs=2, name=f"wo{h}_{grp}_{k}")
                    nc.sync.dma_start(wok[:], w_comb[k * 128:(k + 1) * 128,
                                                     grp * 512:(grp + 1) * 512])
                    for j in range(4):
                        nc.tensor.matmul(psos[j][:], wok[:, j * 128:(j + 1) * 128],
                                         gated[:, k * HALF:(k + 1) * HALF],
                                         start=(k == 0), stop=(k == DT - 1))
                for j in range(4):
                    mo = grp * 4 + j
                    osb = ap_.tile([128, HALF], F32, tag="osb", bufs=1, name=f"osb{h}_{grp}_{j}")
                    nc.scalar.activation(osb[:], psos[j][:], AFT.Identity,
                                         bias=fusb_sb[:, mo:mo + 1])
                    nc.sync.dma_start(part_out[mo * 128:(mo + 1) * 128, hs], osb[:])

        p3p4(0)
        g0 = scan_gate(0)
        p3p4(1)
        p7(0, g0)
        g1 = scan_gate(1)
        p7(1, g1)

    gp.release()
    hp.release()
    xcp.release()
    bigG.release()
    cp.release()


# ---------------------------------------------------------------------------
# Host side
# ---------------------------------------------------------------------------

_NC_CACHE = {}


def _get_nc():
    if "nc" not in _NC_CACHE:
        _NC_CACHE["nc"] = build_bass()
    return _NC_CACHE["nc"]


def _pack_pp(v, ntiles):
    """Pack a (ntiles*128,)-vector into per-partition layout [128, ntiles]."""
    return np.ascontiguousarray(v.reshape(ntiles, 128).T).astype(np.float32)


def make_in_maps(inp):
    x = inp["x"].astype(np.float32)
    ln_g = inp["ln_g"].astype(np.float32)
    ln_b = inp["ln_b"].astype(np.float32)
    fus_w = inp["fus_w"].astype(np.float32)
    fus_b = inp["fus_b"].astype(np.float32)

    in_maps = []
    for ci in range(8):
        d = "f" if ci < 4 else "b"
        b = ci % 4
        x_b = x[b] if d == "f" else x[b][::-1]
        in_w = inp[d + "_in_w"].astype(np.float32)          # (4096, 1024)
        conv_w = inp[d + "_conv_w"].astype(np.float32)      # (2048, 1, 4)
        conv_b = inp[d + "_conv_b"].astype(np.float32)
        xproj_w = inp[d + "_xproj_w"].astype(np.float32)    # (96, 2048)
        dt_w = inp[d + "_dt_w"].astype(np.float32)          # (2048, 64)
        dt_bv = inp[d + "_dt_b"].astype(np.float32)
        A = -np.exp(inp[d + "_A_log"].astype(np.float32))   # (2048, 16)
        Dv = inp[d + "_D"].astype(np.float32)
        out_w = inp[d + "_out_w"].astype(np.float32)        # (1024, 2048)
        wfus = fus_w[:, :D_MODEL] if d == "f" else fus_w[:, D_MODEL:]

        w_in_T = (in_w * ln_g[None, :]).T                   # (1024, 4096)
        cv = in_w @ ln_b                                    # (4096,)
        convw_p = np.zeros((128, DT * D_CONV), np.float32)
        for i in range(DT):
            convw_p[:, i * D_CONV:(i + 1) * D_CONV] = conv_w[i * 128:(i + 1) * 128, 0, :]
        A_p = np.zeros((128, DT * D_STATE), np.float32)
        for i in range(DT):
            A_p[:, i * D_STATE:(i + 1) * D_STATE] = A[i * 128:(i + 1) * 128, :]

        w_cmb = (wfus @ out_w).T                            # (2048, 1024)
        m = {
            "x_t": np.ascontiguousarray(x_b).astype(NPBF),
            "w_in_T": np.ascontiguousarray(w_in_T).astype(NPBF),
            "cvec": _pack_pp(cv, MT),
            "convw": convw_p,
            "convb": _pack_pp(conv_b, DT),
            "w_xproj_T": np.ascontiguousarray(xproj_w.T).astype(NPBF),
            "w_dt_T": np.ascontiguousarray(dt_w.T).astype(NPBF),
            "dt_b": _pack_pp(dt_bv, DT),
            "A_sc": A_p,
            "D_sc": _pack_pp(Dv, DT),
            "w_comb": np.ascontiguousarray(w_cmb).astype(NPBF),
            "fus_b": (_pack_pp(fus_b, DMT) if d == "f"
                      else np.zeros((128, DMT), np.float32)),
        }
        in_maps.append(m)
    return in_maps


def gather(x, results):
    out = np.zeros_like(x)
    for b in range(B_SZ):
        pf = np.asarray(results[b]["part_out"]).T          # (L, D_MODEL)
        pb = np.asarray(results[4 + b]["part_out"]).T[::-1]
        out[b] = pf + pb + x[b]
    return out


def kernel(**inputs):
    inp = {k: np.asarray(v) for k, v in inputs.items()}
    in_maps = make_in_maps(inp)
    from concourse.bass_utils import run_bass_kernel_spmd
    nc = _get_nc()
    res = run_bass_kernel_spmd(nc, in_maps, core_ids=list(range(8)))
    return gather(inp["x"].astype(np.float32), res.results)



# revision 15
# speedup vs baseline: 1.2001x; 1.2001x over previous
"""BiMamba block Trainium2 Bass kernel (8 NeuronCores, SPMD) — v2.

Sharding: 8 cores = 2 directions x 4 batch elements; each core runs the full
Mamba block for one (direction, batch) pair, feature-major, including its
direction's half of the final fused projection (merged via host-precomputed
(fus_w_half @ out_w)). Backward cores consume/produce time-flipped data (host
flips). Host gather: out[b] = partial_f[b].T + flip_L(partial_b[b].T) + x[b].

v2 redesign vs the 1.04 ms baseline (engine-busy driven, TimelineSim):
  - x arrives feature-major; LN runs feature-major: sums over channels via
    PE ones-matmuls, rstd via exp(-0.5 ln var) (stays in the exp/ln ACT
    table), apply as two DVE tensor_tensor with DMA-broadcast stat rows.
    Kills the 64 PE transposes + PSUM evacuations of the baseline.
  - in_proj / out_proj in fp8(e4m3)+DoubleRow: 4x PE throughput; weights
    host-scaled (x16 / x32), un-scaled in the fused evacuation activation.
  - z-gate and conv output use single Silu activations (bias fused) instead
    of sigmoid+copy+mul chains; activation-table loads drop from 42 to ~2
    (exp/ln share one table; silu another).
  - depthwise conv runs on PE as 4 PSUM-accumulated diag(conv_w) matmuls.
  - full-L (1024) scan tiles, no halves: 256 exps/scans of [128,1024]
    instead of 512 of [128,512] (amortizes fixed per-op costs).
  - readout sum over the 16 SSM states runs on PE: identity-weight matmuls
    accumulate C_n*h_n tiles into PSUM (fp32, exact); D*u joins via a
    diag(D) matmul of xc; the gate (*silu(z)) fuses into the PSUM
    evacuation. Removes all readout adds from DVE/GPSIMD.
  - scan a-tiles in fp16 (decay in (0,1]; bf16 quantization was the
    accuracy limit, fp16 is safe); scans are DVE-only (backend rejects
    Pool scans), so the b/Ch elementwise products are split DVE/GPSIMD
    by a tuned ratio instead.
"""

import os
import sys

import numpy as np
import ml_dtypes

for _p in ("/opt/trn_rl_repo", "/root/.axon_site/_ro/trn_rl_repo"):
    if os.path.isdir(_p) and _p not in sys.path:
        sys.path.append(_p)

import concourse.bass as bass
import concourse.mybir as mybir
import concourse.tile as tile
from concourse import bacc
from concourse.masks import make_identity

BF16 = mybir.dt.bfloat16
F16 = mybir.dt.float16
F32 = mybir.dt.float32
F8 = mybir.dt.float8e4
AFT = mybir.ActivationFunctionType
ALU = mybir.AluOpType
MPM = mybir.MatmulPerfMode
NPBF = ml_dtypes.bfloat16
NPF8 = ml_dtypes.float8_e4m3

D_MODEL = 1024
D_STATE = 16
D_CONV = 4
D_INNER = 2048
DT_RANK = 64
B_SZ = 4
L = 1024
LH = 512
LN_EPS = 1e-5
DT = D_INNER // 128           # 16 d-tiles
MT = 2 * D_INNER // 128       # 32 in_proj out tiles
KM = D_MODEL // 128           # 8 k-tiles over d_model
DMT = D_MODEL // 128          # 8 d_model out tiles
SCALE_IN = 16.0               # host multiplies w_in by this (fp8 range)
SCALE_OUT = 32.0              # host multiplies w_comb by this
NBLK = DT // 2                # 8 blocks of 2 d-tiles in the scan phase

# product (bt / Ch) offload: op index % POOL_MOD < POOL_CUT goes to GPSIMD
POOL_MOD = 5
POOL_CUT = 2


def build_bass():
    nc = bacc.Bacc("TRN2", target_bir_lowering=False, debug=False,
                   enable_asserts=False, num_devices=8)

    # ---- DRAM I/O ----
    x_f = nc.dram_tensor("x_f", [D_MODEL, L], BF16, kind="ExternalInput").ap()
    w_in8 = nc.dram_tensor("w_in8", [D_MODEL, 2 * D_INNER], F8, kind="ExternalInput").ap()
    cvec = nc.dram_tensor("cvec", [128, MT], F32, kind="ExternalInput").ap()
    convdiag = nc.dram_tensor("convdiag", [128, DT * D_CONV * 128], BF16, kind="ExternalInput").ap()
    ddiag = nc.dram_tensor("ddiag", [128, DT * 128], BF16, kind="ExternalInput").ap()
    convb = nc.dram_tensor("convb", [128, DT], F32, kind="ExternalInput").ap()
    w_xproj_T = nc.dram_tensor("w_xproj_T", [D_INNER, 96], BF16, kind="ExternalInput").ap()
    w_dt_T = nc.dram_tensor("w_dt_T", [DT_RANK, D_INNER], BF16, kind="ExternalInput").ap()
    dt_b = nc.dram_tensor("dt_b", [128, DT], F32, kind="ExternalInput").ap()
    A_sc = nc.dram_tensor("A_sc", [128, DT * D_STATE], F32, kind="ExternalInput").ap()
    w_comb8 = nc.dram_tensor("w_comb8", [D_INNER, D_MODEL], F8, kind="ExternalInput").ap()
    fus_b = nc.dram_tensor("fus_b", [128, DMT], F32, kind="ExternalInput").ap()
    part_out = nc.dram_tensor("part_out", [D_MODEL, L], F32, kind="ExternalOutput").ap()
    bc_dram = nc.dram_tensor("bc_scratch", [32, L], BF16, kind="Internal").ap()
    g_dram = nc.dram_tensor("g_scratch", [D_INNER, L], BF16, kind="Internal").ap()
    del_dram = nc.dram_tensor("del_scratch", [D_INNER, L], BF16, kind="Internal").ap()
    du_dram = nc.dram_tensor("du_scratch", [D_INNER, L], BF16, kind="Internal").ap()
    row_dram = nc.dram_tensor("row_scratch", [2, L], BF16, kind="Internal").ap()

    with tile.TileContext(nc) as tc:
        _build(tc, x_f, w_in8, cvec, convdiag, ddiag, convb, w_xproj_T, w_dt_T,
               dt_b, A_sc, w_comb8, fus_b, part_out, bc_dram, row_dram, g_dram, del_dram, du_dram)
    nc.compile()
    return nc


def _build(tc, x_f, w_in8, cvec, convdiag, ddiag, convb, w_xproj_T, w_dt_T,
           dt_b, A_sc, w_comb8, fus_b, part_out, bc_dram, row_dram, g_dram,
           del_dram, du_dram):
    nc = tc.nc

    cp = tc.alloc_tile_pool(name="consts", bufs=1)
    ident = cp.tile([128, 128], BF16)
    make_identity(nc, ident)
    ones_col = cp.tile([128, 1], BF16)
    nc.vector.memset(ones_col[:], 1.0)
    one_b = cp.tile([128, 1], F32)
    nc.vector.memset(one_b[:], 1.0)
    eps_b = cp.tile([1, 1], F32)
    nc.vector.memset(eps_b[:], LN_EPS)
    cvec_sb = cp.tile([128, MT], F32)
    convb_sb = cp.tile([128, DT], F32)
    dtb_sb = cp.tile([128, DT], F32)
    A_sb = cp.tile([128, DT * D_STATE], F32)
    fusb_sb = cp.tile([128, DMT], F32)
    wxp = cp.tile([128, DT * 96], BF16)
    wdt = cp.tile([DT_RANK, D_INNER], BF16)
    ddg = cp.tile([128, DT * 128], BF16)

    # resident activations (alloc order = reverse release order)
    gatp = tc.alloc_tile_pool(name="gatp", bufs=1)
    gated = gatp.tile([128, DT * L], F8)              # (y + D*u)*g, fp8 for out_proj
    xcp = tc.alloc_tile_pool(name="xcp", bufs=1)
    xc = xcp.tile([128, DT * L], BF16)                # conv output u
    xip = tc.alloc_tile_pool(name="xip", bufs=1)
    xi = xip.tile([128, DT * (L + 3)], BF16)          # conv input w/ halo

    nc.sync.dma_start(cvec_sb[:], cvec)
    nc.sync.dma_start(convb_sb[:], convb)
    nc.sync.dma_start(dtb_sb[:], dt_b)
    nc.sync.dma_start(A_sb[:], A_sc)
    nc.sync.dma_start(fusb_sb[:], fus_b)
    for k in range(DT):
        nc.sync.dma_start(wxp[:, k * 96:(k + 1) * 96], w_xproj_T[k * 128:(k + 1) * 128, :])
    nc.sync.dma_start(wdt[:], w_dt_T)
    nc.sync.dma_start(ddg[:], ddiag)

    # ================= P0: LayerNorm (feature-major) =================
    xnp_ = tc.alloc_tile_pool(name="xnp", bufs=1)
    xn8 = xnp_.tile([128, KM * L], F8)                # normalized x, fp8 (in_proj rhs)
    with tc.tile_pool(name="p0", bufs=2) as p0, \
         tc.tile_pool(name="p0r", bufs=1) as p0r, \
         tc.tile_pool(name="p0x", bufs=1) as p0x, \
         tc.tile_pool(name="psS", bufs=4, space="PSUM") as psS:
        xt = p0x.tile([128, KM * L], BF16)
        for k in range(KM):
            nc.sync.dma_start(xt[:, k * L:(k + 1) * L], x_f[k * 128:(k + 1) * 128, :])
        # sum(x) and sum(x^2) over channels via ones-matmuls
        sx = [psS.tile([1, LH], F32, tag="sx", name=f"sx{j}") for j in range(2)]
        sxx = [psS.tile([1, LH], F32, tag="sxx", name=f"sxx{j}") for j in range(2)]
        for k in range(KM):
            x2 = p0.tile([128, L], BF16, tag="x2", name=f"x2_{k}")
            nc.scalar.activation(x2[:], xt[:, k * L:(k + 1) * L], AFT.Square)
            for j in range(2):
                nc.tensor.matmul(sx[j][:], ones_col[:], xt[:, k * L + j * LH:k * L + (j + 1) * LH],
                                 start=(k == 0), stop=(k == KM - 1))
                nc.tensor.matmul(sxx[j][:], ones_col[:], x2[:, j * LH:(j + 1) * LH],
                                 start=(k == 0), stop=(k == KM - 1))
        # stats rows ([1, L])
        mu = p0r.tile([1, L], F32, tag="mu")
        ex2 = p0r.tile([1, L], F32, tag="ex2")
        for j in range(2):
            nc.vector.tensor_scalar_mul(mu[:, j * LH:(j + 1) * LH], sx[j][:], 1.0 / D_MODEL)
            nc.vector.tensor_scalar_mul(ex2[:, j * LH:(j + 1) * LH], sxx[j][:], 1.0 / D_MODEL)
        var = p0r.tile([1, L], F32, tag="var")
        nc.vector.tensor_tensor(var[:], mu[:], mu[:], op=ALU.mult)
        nc.vector.tensor_tensor(var[:], ex2[:], var[:], op=ALU.subtract)
        lnv = p0r.tile([1, L], F32, tag="lnv")
        nc.scalar.activation(lnv[:], var[:], AFT.Ln, bias=eps_b[:])
        rstd = p0r.tile([1, L], BF16, tag="rstd")
        nc.scalar.activation(rstd[:], lnv[:], AFT.Exp, scale=-0.5)
        rstd32 = p0r.tile([1, L], F32, tag="rstd32")
        nc.scalar.activation(rstd32[:], lnv[:], AFT.Exp, scale=-0.5)
        mrstd = p0r.tile([1, L], BF16, tag="mrstd")
        nc.vector.tensor_tensor(mrstd[:], mu[:], rstd32[:], op=ALU.mult)
        nc.sync.dma_start(row_dram[0:1, :], rstd[:])
        nc.sync.dma_start(row_dram[1:2, :], mrstd[:])
        rstd_b = p0x.tile([128, L], BF16, tag="rb")
        mrstd_b = p0x.tile([128, L], BF16, tag="mb")
        nc.sync.dma_start(rstd_b[:], row_dram[0:1, :].broadcast_to((128, L)))
        nc.sync.dma_start(mrstd_b[:], row_dram[1:2, :].broadcast_to((128, L)))
        # xn = x*rstd - mu*rstd  (fp8 out)
        for k in range(KM):
            xr = p0.tile([128, L], BF16, tag="xr", name=f"xr{k}")
            nc.vector.tensor_tensor(xr[:], xt[:, k * L:(k + 1) * L], rstd_b[:], op=ALU.mult)
            nc.vector.tensor_tensor(xn8[:, k * L:(k + 1) * L], xr[:], mrstd_b[:], op=ALU.subtract)

    # ================= P1: in_proj (fp8 DoubleRow) =================
    with tc.tile_pool(name="w8p", bufs=1) as w8p, \
         tc.tile_pool(name="psA", bufs=4, space="PSUM") as psA:
        w8 = w8p.tile([128, KM * 2 * D_INNER], F8)
        for k in range(KM):
            nc.sync.dma_start(w8[:, k * 2 * D_INNER:(k + 1) * 2 * D_INNER],
                              w_in8[k * 128:(k + 1) * 128, :])
        for i in range(DT):
            nc.vector.memset(xi[:, i * (L + 3):i * (L + 3) + 3], 0.0)
        WN = 2 * D_INNER
        for m in range(MT):
            for lh in range(2):
                ps = psA.tile([128, LH], F32, tag="mm", name=f"p1_{m}_{lh}")
                for k2 in range(KM // 2):
                    lw = w8[:].rearrange("p (k n) -> p k n", k=KM)[:, 2 * k2:2 * k2 + 2, m * 128:(m + 1) * 128]
                    rh = xn8[:].rearrange("p (k t) -> p k t", k=KM)[:, 2 * k2:2 * k2 + 2, lh * LH:(lh + 1) * LH]
                    nc.tensor.matmul(ps[:], lw, rh, start=(k2 == 0), stop=(k2 == KM // 2 - 1),
                                     perf_mode=MPM.DoubleRow)
                if m < DT:
                    dst = xi[:, m * (L + 3) + 3 + lh * LH: m * (L + 3) + 3 + (lh + 1) * LH]
                    nc.scalar.activation(dst, ps[:], AFT.Identity, bias=cvec_sb[:, m:m + 1],
                                         scale=1.0 / SCALE_IN)
                else:
                    z = m - DT
                    gt = w8p.tile([128, LH], BF16, tag="gt", bufs=3, name=f"gt{m}_{lh}")
                    nc.scalar.activation(gt[:], ps[:], AFT.Silu, bias=cvec_sb[:, m:m + 1],
                                         scale=1.0 / SCALE_IN)
                    nc.sync.dma_start(g_dram[z * 128:(z + 1) * 128, lh * LH:(lh + 1) * LH],
                                      gt[:])
    xnp_.release()

    # ================= P2: depthwise conv on PE =================
    with tc.tile_pool(name="cdp", bufs=1) as cdp, \
         tc.tile_pool(name="psC", bufs=3, space="PSUM") as psC:
        cdg = cdp.tile([128, DT * D_CONV * 128], BF16)
        nc.sync.dma_start(cdg[:], convdiag)
        for i in range(DT):
            base = i * (L + 3) + 3
            for lh in range(2):
                ps = psC.tile([128, LH], F32, tag="cv", name=f"cv{i}_{lh}")
                for tap in range(D_CONV):
                    o = base + lh * LH - (D_CONV - 1 - tap)
                    nc.tensor.matmul(ps[:], cdg[:, (i * D_CONV + tap) * 128:(i * D_CONV + tap + 1) * 128],
                                     xi[:, o:o + LH], start=(tap == 0), stop=(tap == D_CONV - 1))
                nc.scalar.activation(xc[:, i * L + lh * LH:i * L + (lh + 1) * LH],
                                     ps[:], AFT.Silu, bias=convb_sb[:, i:i + 1])
    xip.release()

    # ================= P3: xproj =================
    dtp = tc.alloc_tile_pool(name="dtp", bufs=1)
    dt_sb = dtp.tile([DT_RANK, L], BF16)
    bcp = tc.alloc_tile_pool(name="bcp", bufs=1)
    brep = [bcp.tile([128, L], BF16, name=f"brep{n}") for n in range(D_STATE)]
    crep = [bcp.tile([128, L], BF16, name=f"crep{n}") for n in range(D_STATE)]
    with tc.tile_pool(name="p3", bufs=2) as p3, \
         tc.tile_pool(name="psX", bufs=2, space="PSUM") as psX:
        bc_sb = p3.tile([32, L], BF16, tag="bc")
        for lh in range(2):
            psx = psX.tile([96, LH], F32, tag="xp", name=f"psx{lh}")
            for k in range(DT):
                nc.tensor.matmul(psx[:], wxp[:, k * 96:(k + 1) * 96],
                                 xc[:, k * L + lh * LH:k * L + (lh + 1) * LH],
                                 start=(k == 0), stop=(k == DT - 1))
            nc.scalar.copy(dt_sb[:, lh * LH:(lh + 1) * LH], psx[0:DT_RANK, :])
            nc.scalar.copy(bc_sb[:, lh * LH:(lh + 1) * LH], psx[DT_RANK:96, :])
        nc.sync.dma_start(bc_dram, bc_sb[:])
        for n in range(D_STATE):
            nc.sync.dma_start(brep[n][:], bc_dram[n:n + 1, :].broadcast_to((128, L)))
            nc.sync.dma_start(crep[n][:], bc_dram[D_STATE + n:D_STATE + n + 1, :].broadcast_to((128, L)))

    # ================= P4+P5: dt_proj, softplus, scan =================
    with tc.tile_pool(name="dl", bufs=1) as dl, \
         tc.tile_pool(name="sc", bufs=1) as sc, \
         tc.tile_pool(name="psD", bufs=2, space="PSUM") as psD, \
         tc.tile_pool(name="psY", bufs=6, space="PSUM") as psY:

        prod_idx = [0]

        def prod_engine():
            e = nc.gpsimd if (prod_idx[0] % POOL_MOD) < POOL_CUT else nc.vector
            prod_idx[0] += 1
            return e

        # ---- P4 upfront: dt_proj + softplus for all i (keeps Ln out of P5's
        # pure-Exp ACT stream -> no activation-table thrash); spill to DRAM ----
        for i in range(DT):
            et = dl.tile([128, L], F32, tag="et", bufs=2, name=f"et{i}")
            for lh in range(2):
                psd = psD.tile([128, LH], F32, tag="dt", name=f"psd{i}_{lh}")
                nc.tensor.matmul(psd[:], wdt[:, i * 128:(i + 1) * 128],
                                 dt_sb[:, lh * LH:(lh + 1) * LH], start=True, stop=True)
                nc.scalar.activation(et[:, lh * LH:(lh + 1) * LH], psd[:],
                                     AFT.Exp, bias=dtb_sb[:, i:i + 1])
            delta = dl.tile([128, L], BF16, tag="dm", bufs=3, name=f"dm{i}")
            nc.scalar.activation(delta[:], et[:], AFT.Ln, bias=one_b[:])
            nc.sync.dma_start(del_dram[i * 128:(i + 1) * 128, :], delta[:])
            du = dl.tile([128, L], BF16, tag="du", bufs=3, name=f"du{i}")
            nc.vector.tensor_tensor(du[:], delta[:], xc[:, i * L:(i + 1) * L], op=ALU.mult)
            nc.sync.dma_start(du_dram[i * 128:(i + 1) * 128, :], du[:])

        for blk in range(NBLK):
            i0 = 2 * blk
            gblk = sc.tile([128, 2 * L], BF16, tag="gblk", bufs=2, name=f"gblk{blk}")
            nc.sync.dma_start(gblk[:, 0:L], g_dram[i0 * 128:(i0 + 1) * 128, :])
            nc.sync.dma_start(gblk[:, L:2 * L], g_dram[(i0 + 1) * 128:(i0 + 2) * 128, :])
            dblk = sc.tile([128, 2 * L], BF16, tag="dblk", bufs=2, name=f"dblk{blk}")
            nc.sync.dma_start(dblk[:, 0:L], del_dram[i0 * 128:(i0 + 1) * 128, :])
            nc.sync.dma_start(dblk[:, L:2 * L], del_dram[(i0 + 1) * 128:(i0 + 2) * 128, :])
            dublk = sc.tile([128, 2 * L], BF16, tag="dublk", bufs=2, name=f"dublk{blk}")
            nc.sync.dma_start(dublk[:, 0:L], du_dram[i0 * 128:(i0 + 1) * 128, :])
            nc.sync.dma_start(dublk[:, L:2 * L], du_dram[(i0 + 1) * 128:(i0 + 2) * 128, :])

            # y accumulators (PSUM): per i two L-halves; D*u seeds the sum
            yps = {}
            for ii, i in enumerate((i0, i0 + 1)):
                for lh in range(2):
                    yp = psY.tile([128, LH], F32, tag="y", name=f"y{i}_{lh}")
                    nc.tensor.matmul(yp[:], ddg[:, i * 128:(i + 1) * 128],
                                     xc[:, i * L + lh * LH:i * L + (lh + 1) * LH],
                                     start=True, stop=False)
                    yps[(ii, lh)] = yp

            for n in range(D_STATE):
                bt = sc.tile([128, 2 * L], BF16, tag="bt", bufs=2, name=f"bt{blk}_{n}")
                prod_engine().tensor_tensor(
                    bt[:].rearrange("p (i t) -> p i t", i=2),
                    dublk[:].rearrange("p (i t) -> p i t", i=2),
                    brep[n][:].unsqueeze(1).broadcast_to((128, 2, L)),
                    op=ALU.mult)

                h = sc.tile([128, 2 * L], BF16, tag="h", bufs=2, name=f"h{blk}_{n}")
                for ii in range(2):
                    a = sc.tile([128, L], F16, tag="a", bufs=4, name=f"a{blk}_{n}_{ii}")
                    nc.scalar.activation(a[:], dblk[:, ii * L:(ii + 1) * L], AFT.Exp,
                                         scale=A_sb[:, (i0 + ii) * D_STATE + n:(i0 + ii) * D_STATE + n + 1])
                    nc.vector.tensor_tensor_scan(h[:, ii * L:(ii + 1) * L], a[:],
                                                 bt[:, ii * L:(ii + 1) * L], 0.0,
                                                 op0=ALU.mult, op1=ALU.add)
                ch = sc.tile([128, 2 * L], BF16, tag="ch", bufs=2, name=f"ch{blk}_{n}")
                eng3 = prod_engine()
                eng3.tensor_tensor(
                    ch[:].rearrange("p (i t) -> p i t", i=2),
                    h[:].rearrange("p (i t) -> p i t", i=2),
                    crep[n][:].unsqueeze(1).broadcast_to((128, 2, L)),
                    op=ALU.mult)
                last = (n == D_STATE - 1)
                for ii in range(2):
                    for lh in range(2):
                        nc.tensor.matmul(yps[(ii, lh)][:], ident[:],
                                         ch[:, ii * L + lh * LH:ii * L + (lh + 1) * LH],
                                         start=False, stop=last)
            # gate + evacuate
            for ii, i in enumerate((i0, i0 + 1)):
                for lh in range(2):
                    nc.vector.tensor_tensor(
                        gated[:, i * L + lh * LH:i * L + (lh + 1) * LH],
                        yps[(ii, lh)][:], gblk[:, ii * L + lh * LH:ii * L + (lh + 1) * LH],
                        op=ALU.mult)

    bcp.release()
    dtp.release()
    xcp.release()

    # ================= P7: out_proj (fp8 DoubleRow) =================
    with tc.tile_pool(name="w8o", bufs=1) as w8o, \
         tc.tile_pool(name="p7", bufs=2) as p7, \
         tc.tile_pool(name="psB", bufs=8, space="PSUM") as psB:
        w8c = w8o.tile([128, DT * D_MODEL], F8)
        for k in range(DT):
            nc.sync.dma_start(w8c[:, k * D_MODEL:(k + 1) * D_MODEL],
                              w_comb8[k * 128:(k + 1) * 128, :])
        for lh in range(2):
            pss = [psB.tile([128, LH], F32, tag="o", name=f"o{lh}_{m}") for m in range(DMT)]
            for k2 in range(DT // 2):
                for m in range(DMT):
                    lw = w8c[:].rearrange("p (k n) -> p k n", k=DT)[:, 2 * k2:2 * k2 + 2, m * 128:(m + 1) * 128]
                    rh = gated[:].rearrange("p (k t) -> p k t", k=DT)[:, 2 * k2:2 * k2 + 2, lh * LH:(lh + 1) * LH]
                    nc.tensor.matmul(pss[m][:], lw, rh, start=(k2 == 0), stop=(k2 == DT // 2 - 1),
                                     perf_mode=MPM.DoubleRow)
            for m in range(DMT):
                osb = p7.tile([128, LH], F32, tag="osb", name=f"osb{lh}_{m}")
                nc.scalar.activation(osb[:], pss[m][:], AFT.Identity,
                                     bias=fusb_sb[:, m:m + 1], scale=1.0 / SCALE_OUT)
                nc.sync.dma_start(part_out[m * 128:(m + 1) * 128, lh * LH:(lh + 1) * LH], osb[:])

    gatp.release()
    cp.release()


# ---------------------------------------------------------------------------
# Host side
# ---------------------------------------------------------------------------

_NC_CACHE = {}


def _get_nc():
    if "nc" not in _NC_CACHE:
        _NC_CACHE["nc"] = build_bass()
    return _NC_CACHE["nc"]


def _pack_pp(v, ntiles):
    """Pack a (ntiles*128,)-vector into per-partition layout [128, ntiles]."""
    return np.ascontiguousarray(v.reshape(ntiles, 128).T).astype(np.float32)


def make_in_maps(inp):
    x = inp["x"].astype(np.float32)
    ln_g = inp["ln_g"].astype(np.float32)
    ln_b = inp["ln_b"].astype(np.float32)
    fus_w = inp["fus_w"].astype(np.float32)
    fus_b = inp["fus_b"].astype(np.float32)

    in_maps = []
    for ci in range(8):
        d = "f" if ci < 4 else "b"
        b = ci % 4
        x_b = x[b] if d == "f" else x[b][::-1]
        in_w = inp[d + "_in_w"].astype(np.float32)          # (4096, 1024)
        conv_w = inp[d + "_conv_w"].astype(np.float32)      # (2048, 1, 4)
        conv_b = inp[d + "_conv_b"].astype(np.float32)
        xproj_w = inp[d + "_xproj_w"].astype(np.float32)    # (96, 2048)
        dt_w = inp[d + "_dt_w"].astype(np.float32)          # (2048, 64)
        dt_bv = inp[d + "_dt_b"].astype(np.float32)
        A = -np.exp(inp[d + "_A_log"].astype(np.float32))   # (2048, 16)
        Dv = inp[d + "_D"].astype(np.float32)
        out_w = inp[d + "_out_w"].astype(np.float32)        # (1024, 2048)
        wfus = fus_w[:, :D_MODEL] if d == "f" else fus_w[:, D_MODEL:]

        w_in_T = (in_w * ln_g[None, :]).T                   # (1024, 4096)
        cv = in_w @ ln_b                                    # (4096,)
        convdiag = np.zeros((128, DT * D_CONV * 128), np.float32)
        for i in range(DT):
            for k in range(D_CONV):
                blkw = np.diag(conv_w[i * 128:(i + 1) * 128, 0, k])
                convdiag[:, (i * D_CONV + k) * 128:(i * D_CONV + k + 1) * 128] = blkw
        ddiag = np.zeros((128, DT * 128), np.float32)
        for i in range(DT):
            ddiag[:, i * 128:(i + 1) * 128] = np.diag(Dv[i * 128:(i + 1) * 128])
        A_p = np.zeros((128, DT * D_STATE), np.float32)
        for i in range(DT):
            A_p[:, i * D_STATE:(i + 1) * D_STATE] = A[i * 128:(i + 1) * 128, :]

        w_cmb = (wfus @ out_w).T                            # (2048, 1024)
        m = {
            "x_f": np.ascontiguousarray(x_b.T).astype(NPBF),
            "w_in8": np.ascontiguousarray(w_in_T * SCALE_IN).astype(NPF8),
            "cvec": _pack_pp(cv, MT),
            "convdiag": convdiag.astype(NPBF),
            "ddiag": ddiag.astype(NPBF),
            "convb": _pack_pp(conv_b, DT),
            "w_xproj_T": np.ascontiguousarray(xproj_w.T).astype(NPBF),
            "w_dt_T": np.ascontiguousarray(dt_w.T).astype(NPBF),
            "dt_b": _pack_pp(dt_bv, DT),
            "A_sc": A_p,
            "w_comb8": np.ascontiguousarray(w_cmb * SCALE_OUT).astype(NPF8),
            "fus_b": (_pack_pp(fus_b, DMT) if d == "f"
                      else np.zeros((128, DMT), np.float32)),
        }
        in_maps.append(m)
    return in_maps


def gather(x, results):
    out = np.zeros_like(x)
    for b in range(B_SZ):
        pf = np.asarray(results[b]["part_out"]).T          # (L, D_MODEL)
        pb = np.asarray(results[4 + b]["part_out"]).T[::-1]
        out[b] = pf + pb + x[b]
    return out


def kernel(**inputs):
    inp = {k: np.asarray(v) for k, v in inputs.items()}
    in_maps = make_in_maps(inp)
    from concourse.bass_utils import run_bass_kernel_spmd
    nc = _get_nc()
    res = run_bass_kernel_spmd(nc, in_maps, core_ids=list(range(8)))
    return gather(inp["x"].astype(np.float32), res.results)


# revision 31
# speedup vs baseline: 1.4845x; 1.2369x over previous
"""BiMamba block Trainium2 Bass kernel (8 NeuronCores, SPMD) — v2.

Sharding: 8 cores = 2 directions x 4 batch elements; each core runs the full
Mamba block for one (direction, batch) pair, feature-major, including its
direction's half of the final fused projection (merged via host-precomputed
(fus_w_half @ out_w)). Backward cores consume/produce time-flipped data (host
flips). Host gather: out[b] = partial_f[b].T + flip_L(partial_b[b].T) + x[b].

v2 redesign vs the 1.04 ms baseline (engine-busy driven, TimelineSim):
  - x arrives feature-major; LN runs feature-major: sums over channels via
    PE ones-matmuls, rstd via exp(-0.5 ln var) (stays in the exp/ln ACT
    table), apply as two DVE tensor_tensor with DMA-broadcast stat rows.
    Kills the 64 PE transposes + PSUM evacuations of the baseline.
  - in_proj / out_proj in fp8(e4m3)+DoubleRow: 4x PE throughput; weights
    host-scaled (x16 / x32), un-scaled in the fused evacuation activation.
  - z-gate and conv output use single Silu activations (bias fused) instead
    of sigmoid+copy+mul chains; activation-table loads drop from 42 to ~2
    (exp/ln share one table; silu another).
  - depthwise conv runs on PE as 4 PSUM-accumulated diag(conv_w) matmuls.
  - full-L (1024) scan tiles, no halves: 256 exps/scans of [128,1024]
    instead of 512 of [128,512] (amortizes fixed per-op costs).
  - readout sum over the 16 SSM states runs on PE: identity-weight matmuls
    accumulate C_n*h_n tiles into PSUM (fp32, exact); D*u joins via a
    diag(D) matmul of xc; the gate (*silu(z)) fuses into the PSUM
    evacuation. Removes all readout adds from DVE/GPSIMD.
  - scan a-tiles in fp16 (decay in (0,1]; bf16 quantization was the
    accuracy limit, fp16 is safe); scans are DVE-only (backend rejects
    Pool scans), so the b/Ch elementwise products are split DVE/GPSIMD
    by a tuned ratio instead.
"""

import os
import sys

import numpy as np
import ml_dtypes

for _p in ("/opt/trn_rl_repo", "/root/.axon_site/_ro/trn_rl_repo"):
    if os.path.isdir(_p) and _p not in sys.path:
        sys.path.append(_p)

import concourse.bass as bass
import concourse.mybir as mybir
import concourse.tile as tile
from concourse import bacc
from concourse.masks import make_identity

BF16 = mybir.dt.bfloat16
F16 = mybir.dt.float16
F32 = mybir.dt.float32
F8 = mybir.dt.float8e4
AFT = mybir.ActivationFunctionType
ALU = mybir.AluOpType
MPM = mybir.MatmulPerfMode
NPBF = ml_dtypes.bfloat16
NPF8 = ml_dtypes.float8_e4m3

D_MODEL = 1024
D_STATE = 16
D_CONV = 4
D_INNER = 2048
DT_RANK = 64
B_SZ = 4
L = 1024
LH = 512
LN_EPS = 1e-5
DT = D_INNER // 128           # 16 d-tiles
MT = 2 * D_INNER // 128       # 32 in_proj out tiles
KM = D_MODEL // 128           # 8 k-tiles over d_model
DMT = D_MODEL // 128          # 8 d_model out tiles
SCALE_IN = 16.0               # host multiplies w_in by this (fp8 range)
SCALE_OUT = 32.0              # host multiplies w_comb by this
NBLK = DT // 2                # 8 blocks of 2 d-tiles in the scan phase

# Ch offload: ch op index % CH_POOL_MOD < CH_POOL_CUT goes to GPSIMD
# (bt stays on DVE: a Pool bt would head-of-line block the scans)
CH_POOL_MOD = 6
CH_POOL_CUT = 5


def _prefer_exp_ln_table():
    """Reorder the (cached) activation-table dict so the table containing BOTH
    exp and ln is preferred by the greedy table chooser. Otherwise every
    ln->exp transition in the ACT stream pays a 1.3us table load (the chooser
    takes the first table containing the function, and the ln-only table
    precedes the exp+ln one in act_info order)."""
    try:
        from concourse.hw_specs import get_activation_tables
        tabs = get_activation_tables("gen3")
        has_both = [k for k, v in tabs.items()
                    if any(f.name == "Exp" for f in v) and any(f.name == "Ln" for f in v)]
        if not has_both:
            return
        # Keep dict ORDER intact (act_func_set_id is the canonical index that
        # walrus also uses); instead drop Exp/Ln from single-function tables so
        # the greedy chooser can only pick the combined exp+ln table.
        for k, v in tabs.items():
            if k in has_both:
                continue
            for f in list(v):
                if f.name in ("Exp", "Ln"):
                    v.discard(f)
    except Exception:
        pass


def build_bass():
    _prefer_exp_ln_table()
    nc = bacc.Bacc("TRN2", target_bir_lowering=False, debug=False,
                   enable_asserts=False, num_devices=8)

    # ---- DRAM I/O ----
    x_f = nc.dram_tensor("x_f", [D_MODEL, L], BF16, kind="ExternalInput").ap()
    w_in8 = nc.dram_tensor("w_in8", [D_MODEL, 2 * D_INNER], F8, kind="ExternalInput").ap()
    cvec = nc.dram_tensor("cvec", [128, MT], F32, kind="ExternalInput").ap()
    convdiag = nc.dram_tensor("convdiag", [128, DT * D_CONV * 128], BF16, kind="ExternalInput").ap()
    ddiag = nc.dram_tensor("ddiag", [128, DT * 128], BF16, kind="ExternalInput").ap()
    convb = nc.dram_tensor("convb", [128, DT], F32, kind="ExternalInput").ap()
    w_xproj_T = nc.dram_tensor("w_xproj_T", [D_INNER, 96], BF16, kind="ExternalInput").ap()
    w_dt_T = nc.dram_tensor("w_dt_T", [DT_RANK, D_INNER], BF16, kind="ExternalInput").ap()
    dt_b = nc.dram_tensor("dt_b", [128, DT], F32, kind="ExternalInput").ap()
    A_sc = nc.dram_tensor("A_sc", [128, DT * D_STATE], F32, kind="ExternalInput").ap()
    w_comb8 = nc.dram_tensor("w_comb8", [D_INNER, D_MODEL], F8, kind="ExternalInput").ap()
    fus_b = nc.dram_tensor("fus_b", [128, DMT], F32, kind="ExternalInput").ap()
    part_out = nc.dram_tensor("part_out", [D_MODEL, L], F32, kind="ExternalOutput").ap()
    bc_dram = nc.dram_tensor("bc_scratch", [32, L], BF16, kind="Internal").ap()
    g_dram = nc.dram_tensor("g_scratch", [D_INNER, L], BF16, kind="Internal").ap()
    row_dram = nc.dram_tensor("row_scratch", [2, L], BF16, kind="Internal").ap()

    with tile.TileContext(nc) as tc:
        _build(tc, x_f, w_in8, cvec, convdiag, ddiag, convb, w_xproj_T, w_dt_T,
               dt_b, A_sc, w_comb8, fus_b, part_out, bc_dram, row_dram, g_dram)
    nc.compile()
    return nc


def _build(tc, x_f, w_in8, cvec, convdiag, ddiag, convb, w_xproj_T, w_dt_T,
           dt_b, A_sc, w_comb8, fus_b, part_out, bc_dram, row_dram, g_dram):
    nc = tc.nc

    cp = tc.alloc_tile_pool(name="consts", bufs=1)
    ident = cp.tile([128, 128], BF16)
    make_identity(nc, ident)
    ones_col = cp.tile([128, 1], BF16)
    nc.vector.memset(ones_col[:], 1.0)
    one_b = cp.tile([128, 1], F32)
    nc.vector.memset(one_b[:], 1.0)
    eps_b = cp.tile([1, 1], F32)
    nc.vector.memset(eps_b[:], LN_EPS)
    cvec_sb = cp.tile([128, MT], F32)
    convb_sb = cp.tile([128, DT], F32)
    dtb_sb = cp.tile([128, DT], F32)
    A_sb = cp.tile([128, DT * D_STATE], F32)
    fusb_sb = cp.tile([128, DMT], F32)
    wxp = cp.tile([128, DT * 96], BF16)
    wdt = cp.tile([DT_RANK, D_INNER], BF16)
    ddg = cp.tile([128, DT * 128], BF16)

    # resident activations (alloc order = reverse release order)
    gatp = tc.alloc_tile_pool(name="gatp", bufs=1)
    gated = gatp.tile([128, DT * L], F8)              # (y + D*u)*g, fp8 for out_proj
    xcp = tc.alloc_tile_pool(name="xcp", bufs=1)
    xc = xcp.tile([128, DT * L], BF16)                # conv output u
    xip = tc.alloc_tile_pool(name="xip", bufs=1)
    xi = xip.tile([128, DT * (L + 3)], BF16)          # conv input w/ halo

    nc.sync.dma_start(cvec_sb[:], cvec)
    nc.sync.dma_start(convb_sb[:], convb)
    nc.sync.dma_start(dtb_sb[:], dt_b)
    nc.sync.dma_start(A_sb[:], A_sc)
    nc.sync.dma_start(fusb_sb[:], fus_b)
    for k in range(DT):
        nc.sync.dma_start(wxp[:, k * 96:(k + 1) * 96], w_xproj_T[k * 128:(k + 1) * 128, :])
    nc.sync.dma_start(wdt[:], w_dt_T)
    nc.sync.dma_start(ddg[:], ddiag)

    # ================= P0-P2: LN + in_proj + conv, pipelined by L-half ====
    xnp_ = tc.alloc_tile_pool(name="xnp", bufs=1)
    xn8 = xnp_.tile([128, KM * L], F8)                # normalized x, fp8 (in_proj rhs)
    with tc.tile_pool(name="p0", bufs=2) as p0, \
         tc.tile_pool(name="p0r", bufs=1) as p0r, \
         tc.tile_pool(name="p0x", bufs=1) as p0x, \
         tc.tile_pool(name="w8p", bufs=1) as w8p, \
         tc.tile_pool(name="cdp", bufs=4) as cdp:
      with tc.tile_pool(name="psS", bufs=1, space="PSUM") as psS, \
           tc.tile_pool(name="psA", bufs=4, space="PSUM") as psA, \
           tc.tile_pool(name="psC", bufs=2, space="PSUM") as psC:
        xt = p0x.tile([128, KM * L], BF16)
        w8 = w8p.tile([128, KM * 2 * D_INNER], F8)
        for k in range(KM):
            nc.sync.dma_start(w8[:, k * 2 * D_INNER:(k + 1) * 2 * D_INNER],
                              w_in8[k * 128:(k + 1) * 128, :])
        for i in range(DT):
            nc.vector.memset(xi[:, i * (L + 3):i * (L + 3) + 3], 0.0)
        for lh in range(2):
            hs = slice(lh * LH, (lh + 1) * LH)
            for k in range(KM):
                nc.sync.dma_start(xt[:, k * L + lh * LH:k * L + (lh + 1) * LH],
                                  x_f[k * 128:(k + 1) * 128, hs])
            # LN stats for this half (channel sums via ones-matmuls)
            sx = psS.tile([1, LH], F32, tag="sx", name=f"sx{lh}")
            sxx = psS.tile([1, LH], F32, tag="sxx", name=f"sxx{lh}")
            for k in range(KM):
                x2 = p0.tile([128, LH], BF16, tag="x2", name=f"x2_{lh}_{k}")
                nc.scalar.activation(x2[:], xt[:, k * L + lh * LH:k * L + (lh + 1) * LH], AFT.Square)
                nc.tensor.matmul(sx[:], ones_col[:], xt[:, k * L + lh * LH:k * L + (lh + 1) * LH],
                                 start=(k == 0), stop=(k == KM - 1))
                nc.tensor.matmul(sxx[:], ones_col[:], x2[:], start=(k == 0), stop=(k == KM - 1))
            mu = p0r.tile([1, LH], F32, tag="mu", bufs=2, name=f"mu{lh}")
            ex2 = p0r.tile([1, LH], F32, tag="ex2", bufs=2, name=f"ex2{lh}")
            nc.vector.tensor_scalar_mul(mu[:], sx[:], 1.0 / D_MODEL)
            nc.vector.tensor_scalar_mul(ex2[:], sxx[:], 1.0 / D_MODEL)
            var = p0r.tile([1, LH], F32, tag="var", bufs=2, name=f"var{lh}")
            nc.vector.tensor_tensor(var[:], mu[:], mu[:], op=ALU.mult)
            nc.vector.tensor_tensor(var[:], ex2[:], var[:], op=ALU.subtract)
            lnv = p0r.tile([1, LH], F32, tag="lnv", bufs=2, name=f"lnv{lh}")
            nc.scalar.activation(lnv[:], var[:], AFT.Ln, bias=eps_b[:])
            rstd = p0r.tile([1, LH], BF16, tag="rstd", bufs=2, name=f"rstd{lh}")
            nc.scalar.activation(rstd[:], lnv[:], AFT.Exp, scale=-0.5)
            rstd32 = p0r.tile([1, LH], F32, tag="rstd32", bufs=2, name=f"rstd32{lh}")
            nc.scalar.activation(rstd32[:], lnv[:], AFT.Exp, scale=-0.5)
            mrstd = p0r.tile([1, LH], BF16, tag="mrstd", bufs=2, name=f"mrstd{lh}")
            nc.vector.tensor_tensor(mrstd[:], mu[:], rstd32[:], op=ALU.mult)
            nc.sync.dma_start(row_dram[0:1, hs], rstd[:])
            nc.sync.dma_start(row_dram[1:2, hs], mrstd[:])
            rstd_b = p0x.tile([128, LH], BF16, tag="rb", bufs=2, name=f"rb{lh}")
            mrstd_b = p0x.tile([128, LH], BF16, tag="mb", bufs=2, name=f"mb{lh}")
            nc.sync.dma_start(rstd_b[:], row_dram[0:1, hs].broadcast_to((128, LH)))
            nc.sync.dma_start(mrstd_b[:], row_dram[1:2, hs].broadcast_to((128, LH)))
            for k in range(KM):
                xr = p0.tile([128, LH], BF16, tag="xr", name=f"xr{lh}_{k}")
                nc.vector.tensor_tensor(xr[:], xt[:, k * L + lh * LH:k * L + (lh + 1) * LH],
                                        rstd_b[:], op=ALU.mult)
                nc.vector.tensor_tensor(xn8[:, k * L + lh * LH:k * L + (lh + 1) * LH],
                                        xr[:], mrstd_b[:], op=ALU.subtract)
            # in_proj xi-half for this L-half
            for m in range(DT):
                ps = psA.tile([128, LH], F32, tag="mm", name=f"p1_{m}_{lh}")
                for k2 in range(KM // 2):
                    lw = w8[:].rearrange("p (k n) -> p k n", k=KM)[:, 2 * k2:2 * k2 + 2, m * 128:(m + 1) * 128]
                    rh = xn8[:].rearrange("p (k t) -> p k t", k=KM)[:, 2 * k2:2 * k2 + 2, lh * LH:(lh + 1) * LH]
                    nc.tensor.matmul(ps[:], lw, rh, start=(k2 == 0), stop=(k2 == KM // 2 - 1),
                                     perf_mode=MPM.DoubleRow)
                dst = xi[:, m * (L + 3) + 3 + lh * LH: m * (L + 3) + 3 + (lh + 1) * LH]
                nc.vector.tensor_scalar(dst, ps[:], 1.0 / SCALE_IN, cvec_sb[:, m:m + 1],
                                        op0=ALU.mult, op1=ALU.add)
            # conv for this L-half
            for i in range(DT):
                base = i * (L + 3) + 3
                cdg = cdp.tile([128, D_CONV * 128], BF16, tag="cdg", name=f"cdg{lh}_{i}")
                nc.sync.dma_start(cdg[:], convdiag[:, i * D_CONV * 128:(i + 1) * D_CONV * 128])
                ps = psC.tile([128, LH], F32, tag="cv", name=f"cv{i}_{lh}")
                for tap in range(D_CONV):
                    o = base + lh * LH - (D_CONV - 1 - tap)
                    nc.tensor.matmul(ps[:], cdg[:, tap * 128:(tap + 1) * 128],
                                     xi[:, o:o + LH], start=(tap == 0), stop=(tap == D_CONV - 1))
                nc.scalar.activation(xc[:, i * L + lh * LH:i * L + (lh + 1) * LH],
                                     ps[:], AFT.Silu, bias=convb_sb[:, i:i + 1])
      # z-half of in_proj (feeds only the P5 gate; emitted last, and the
      # xproj matmuls below preempt these on PE via high_priority)
      if True:
        with tc.tile_pool(name="psZ", bufs=2, space="PSUM") as psZ:
            for m in range(DT, MT):
                ps = psZ.tile([128, L], F32, tag="zz", name=f"p1z_{m}")
                for lh in range(2):
                    for k2 in range(KM // 2):
                        lw = w8[:].rearrange("p (k n) -> p k n", k=KM)[:, 2 * k2:2 * k2 + 2, m * 128:(m + 1) * 128]
                        rh = xn8[:].rearrange("p (k t) -> p k t", k=KM)[:, 2 * k2:2 * k2 + 2, lh * LH:(lh + 1) * LH]
                        nc.tensor.matmul(ps[:, lh * LH:(lh + 1) * LH], lw, rh,
                                         start=(k2 == 0), stop=(k2 == KM // 2 - 1),
                                         perf_mode=MPM.DoubleRow)
                z = m - DT
                gt = w8p.tile([128, L], BF16, tag="gt", bufs=3, name=f"gt{m}")
                nc.scalar.activation(gt[:], ps[:], AFT.Silu, bias=cvec_sb[:, m:m + 1],
                                     scale=1.0 / SCALE_IN)
                nc.sync.dma_start(g_dram[z * 128:(z + 1) * 128, :], gt[:])
    xnp_.release()
    xip.release()

    # out_proj weights: allocate + DMA early so the fetch hides under P5
    w8o = tc.alloc_tile_pool(name="w8o", bufs=1)
    w8c = w8o.tile([128, DT * D_MODEL], F8)
    for k in range(DT):
        nc.sync.dma_start(w8c[:, k * D_MODEL:(k + 1) * D_MODEL],
                          w_comb8[k * 128:(k + 1) * 128, :])

    # ================= P3: xproj =================
    dtp = tc.alloc_tile_pool(name="dtp", bufs=1)
    dt_sb = dtp.tile([DT_RANK, L], BF16)
    bcp = tc.alloc_tile_pool(name="bcp", bufs=1)
    brep = [bcp.tile([128, L], BF16, name=f"brep{n}") for n in range(D_STATE)]
    crep = [bcp.tile([128, L], BF16, name=f"crep{n}") for n in range(D_STATE)]
    with tc.tile_pool(name="p3", bufs=2) as p3, \
         tc.tile_pool(name="psX", bufs=2, space="PSUM") as psX:
        bc_sb = p3.tile([32, L], BF16, tag="bc")
        with tc.high_priority(offset=600):
            for lh in range(2):
                psx = psX.tile([96, LH], F32, tag="xp", name=f"psx{lh}")
                for k in range(DT):
                    nc.tensor.matmul(psx[:], wxp[:, k * 96:(k + 1) * 96],
                                     xc[:, k * L + lh * LH:k * L + (lh + 1) * LH],
                                     start=(k == 0), stop=(k == DT - 1))
                nc.scalar.copy(dt_sb[:, lh * LH:(lh + 1) * LH], psx[0:DT_RANK, :])
                nc.scalar.copy(bc_sb[:, lh * LH:(lh + 1) * LH], psx[DT_RANK:96, :])
        nc.sync.dma_start(bc_dram, bc_sb[:])
        for n in range(D_STATE):
            nc.sync.dma_start(brep[n][:], bc_dram[n:n + 1, :].broadcast_to((128, L)))
            nc.sync.dma_start(crep[n][:], bc_dram[D_STATE + n:D_STATE + n + 1, :].broadcast_to((128, L)))

    # ================= P4+P5: dt_proj, softplus, scan =================
    with tc.tile_pool(name="dl", bufs=1) as dl, \
         tc.tile_pool(name="sc", bufs=1) as sc, \
         tc.tile_pool(name="psD", bufs=2, space="PSUM") as psD, \
         tc.tile_pool(name="psY", bufs=6, space="PSUM") as psY:

        ch_idx = [0]

        def ch_engine():
            e = nc.gpsimd if (ch_idx[0] % CH_POOL_MOD) < CH_POOL_CUT else nc.vector
            ch_idx[0] += 1
            return e

        for blk in range(NBLK):
            i0 = 2 * blk
            gblk = sc.tile([128, 2 * L], BF16, tag="gblk", bufs=2, name=f"gblk{blk}")
            nc.sync.dma_start(gblk[:, 0:L], g_dram[i0 * 128:(i0 + 1) * 128, :])
            nc.sync.dma_start(gblk[:, L:2 * L], g_dram[(i0 + 1) * 128:(i0 + 2) * 128, :])
            # per-block P4: dt_proj + softplus (exp/ln share the preferred
            # activation table with the scan exps -> no table loads)
            dblk = sc.tile([128, 2 * L], BF16, tag="dblk", bufs=2, name=f"dblk{blk}")
            dublk = sc.tile([128, 2 * L], BF16, tag="dublk", bufs=2, name=f"dublk{blk}")
            for ii, i in enumerate((i0, i0 + 1)):
                et = dl.tile([128, L], BF16, tag="et", bufs=2, name=f"et{i}")
                for lh in range(2):
                    psd = psD.tile([128, LH], F32, tag="dt", name=f"psd{i}_{lh}")
                    nc.tensor.matmul(psd[:], wdt[:, i * 128:(i + 1) * 128],
                                     dt_sb[:, lh * LH:(lh + 1) * LH], start=True, stop=True)
                    nc.scalar.activation(et[:, lh * LH:(lh + 1) * LH], psd[:],
                                         AFT.Exp, bias=dtb_sb[:, i:i + 1])
                nc.scalar.activation(dblk[:, ii * L:(ii + 1) * L], et[:], AFT.Ln, bias=one_b[:])
            nc.vector.tensor_tensor(dublk[:], dblk[:], xc[:, i0 * L:(i0 + 2) * L], op=ALU.mult)

            # y accumulators (PSUM): per i two L-halves; D*u seeds the sum
            yps = {}
            for ii, i in enumerate((i0, i0 + 1)):
                for lh in range(2):
                    yp = psY.tile([128, LH], F32, tag="y", name=f"y{i}_{lh}")
                    nc.tensor.matmul(yp[:], ddg[:, i * 128:(i + 1) * 128],
                                     xc[:, i * L + lh * LH:i * L + (lh + 1) * LH],
                                     start=True, stop=False)
                    yps[(ii, lh)] = yp

            for n in range(D_STATE):
                bt = sc.tile([128, 2 * L], BF16, tag="bt", bufs=2, name=f"bt{blk}_{n}")
                nc.vector.tensor_tensor(
                    bt[:].rearrange("p (i t) -> p i t", i=2),
                    dublk[:].rearrange("p (i t) -> p i t", i=2),
                    brep[n][:].unsqueeze(1).broadcast_to((128, 2, L)),
                    op=ALU.mult)

                h = sc.tile([128, 2 * L], BF16, tag="h", bufs=3, name=f"h{blk}_{n}")
                for ii in range(2):
                    a = sc.tile([128, L], F16, tag="a", bufs=2, name=f"a{blk}_{n}_{ii}")
                    nc.scalar.activation(a[:], dblk[:, ii * L:(ii + 1) * L], AFT.Exp,
                                         scale=A_sb[:, (i0 + ii) * D_STATE + n:(i0 + ii) * D_STATE + n + 1])
                    nc.vector.tensor_tensor_scan(h[:, ii * L:(ii + 1) * L], a[:],
                                                 bt[:, ii * L:(ii + 1) * L], 0.0,
                                                 op0=ALU.mult, op1=ALU.add)
                ch = sc.tile([128, 2 * L], BF16, tag="ch", bufs=3, name=f"ch{blk}_{n}")
                ch_engine().tensor_tensor(
                    ch[:].rearrange("p (i t) -> p i t", i=2),
                    h[:].rearrange("p (i t) -> p i t", i=2),
                    crep[n][:].unsqueeze(1).broadcast_to((128, 2, L)),
                    op=ALU.mult)
                last = (n == D_STATE - 1)
                for ii in range(2):
                    for lh in range(2):
                        nc.tensor.matmul(yps[(ii, lh)][:], ident[:],
                                         ch[:, ii * L + lh * LH:ii * L + (lh + 1) * LH],
                                         start=False, stop=last)
            # gate + evacuate
            for ii, i in enumerate((i0, i0 + 1)):
                for lh in range(2):
                    nc.vector.tensor_tensor(
                        gated[:, i * L + lh * LH:i * L + (lh + 1) * LH],
                        yps[(ii, lh)][:], gblk[:, ii * L + lh * LH:ii * L + (lh + 1) * LH],
                        op=ALU.mult)

    bcp.release()
    dtp.release()

    # ================= P7: out_proj (fp8 DoubleRow) =================
    with tc.tile_pool(name="p7", bufs=2) as p7, \
         tc.tile_pool(name="psB", bufs=8, space="PSUM") as psB:
        for lh in range(2):
            pss = [psB.tile([128, LH], F32, tag="o", name=f"o{lh}_{m}") for m in range(DMT)]
            for k2 in range(DT // 2):
                for m in range(DMT):
                    lw = w8c[:].rearrange("p (k n) -> p k n", k=DT)[:, 2 * k2:2 * k2 + 2, m * 128:(m + 1) * 128]
                    rh = gated[:].rearrange("p (k t) -> p k t", k=DT)[:, 2 * k2:2 * k2 + 2, lh * LH:(lh + 1) * LH]
                    nc.tensor.matmul(pss[m][:], lw, rh, start=(k2 == 0), stop=(k2 == DT // 2 - 1),
                                     perf_mode=MPM.DoubleRow)
            for m in range(DMT):
                osb = p7.tile([128, LH], F32, tag="osb", name=f"osb{lh}_{m}")
                nc.scalar.activation(osb[:], pss[m][:], AFT.Identity,
                                     bias=fusb_sb[:, m:m + 1], scale=1.0 / SCALE_OUT)
                nc.sync.dma_start(part_out[m * 128:(m + 1) * 128, lh * LH:(lh + 1) * LH], osb[:])

    w8o.release()
    xcp.release()
    gatp.release()
    cp.release()


# ---------------------------------------------------------------------------
# Host side
# ---------------------------------------------------------------------------

_NC_CACHE = {}


def _get_nc():
    if "nc" not in _NC_CACHE:
        _NC_CACHE["nc"] = build_bass()
    return _NC_CACHE["nc"]


def _pack_pp(v, ntiles):
    """Pack a (ntiles*128,)-vector into per-partition layout [128, ntiles]."""
    return np.ascontiguousarray(v.reshape(ntiles, 128).T).astype(np.float32)


def make_in_maps(inp):
    x = inp["x"].astype(np.float32)
    ln_g = inp["ln_g"].astype(np.float32)
    ln_b = inp["ln_b"].astype(np.float32)
    fus_w = inp["fus_w"].astype(np.float32)
    fus_b = inp["fus_b"].astype(np.float32)

    in_maps = []
    for ci in range(8):
        d = "f" if ci < 4 else "b"
        b = ci % 4
        x_b = x[b] if d == "f" else x[b][::-1]
        in_w = inp[d + "_in_w"].astype(np.float32)          # (4096, 1024)
        conv_w = inp[d + "_conv_w"].astype(np.float32)      # (2048, 1, 4)
        conv_b = inp[d + "_conv_b"].astype(np.float32)
        xproj_w = inp[d + "_xproj_w"].astype(np.float32)    # (96, 2048)
        dt_w = inp[d + "_dt_w"].astype(np.float32)          # (2048, 64)
        dt_bv = inp[d + "_dt_b"].astype(np.float32)
        A = -np.exp(inp[d + "_A_log"].astype(np.float32))   # (2048, 16)
        Dv = inp[d + "_D"].astype(np.float32)
        out_w = inp[d + "_out_w"].astype(np.float32)        # (1024, 2048)
        wfus = fus_w[:, :D_MODEL] if d == "f" else fus_w[:, D_MODEL:]

        w_in_T = (in_w * ln_g[None, :]).T                   # (1024, 4096)
        cv = in_w @ ln_b                                    # (4096,)
        convdiag = np.zeros((128, DT * D_CONV * 128), np.float32)
        for i in range(DT):
            for k in range(D_CONV):
                blkw = np.diag(conv_w[i * 128:(i + 1) * 128, 0, k])
                convdiag[:, (i * D_CONV + k) * 128:(i * D_CONV + k + 1) * 128] = blkw
        ddiag = np.zeros((128, DT * 128), np.float32)
        for i in range(DT):
            ddiag[:, i * 128:(i + 1) * 128] = np.diag(Dv[i * 128:(i + 1) * 128])
        A_p = np.zeros((128, DT * D_STATE), np.float32)
        for i in range(DT):
            A_p[:, i * D_STATE:(i + 1) * D_STATE] = A[i * 128:(i + 1) * 128, :]

        w_cmb = (wfus @ out_w).T                            # (2048, 1024)
        m = {
            "x_f": np.ascontiguousarray(x_b.T).astype(NPBF),
            "w_in8": np.ascontiguousarray(w_in_T * SCALE_IN).astype(NPF8),
            "cvec": _pack_pp(cv, MT),
            "convdiag": convdiag.astype(NPBF),
            "ddiag": ddiag.astype(NPBF),
            "convb": _pack_pp(conv_b, DT),
            "w_xproj_T": np.ascontiguousarray(xproj_w.T).astype(NPBF),
            "w_dt_T": np.ascontiguousarray(dt_w.T).astype(NPBF),
            "dt_b": _pack_pp(dt_bv, DT),
            "A_sc": A_p,
            "w_comb8": np.ascontiguousarray(w_cmb * SCALE_OUT).astype(NPF8),
            "fus_b": (_pack_pp(fus_b, DMT) if d == "f"
                      else np.zeros((128, DMT), np.float32)),
        }
        in_maps.append(m)
    return in_maps


def gather(x, results):
    out = np.zeros_like(x)
    for b in range(B_SZ):
        pf = np.asarray(results[b]["part_out"]).T          # (L, D_MODEL)
        pb = np.asarray(results[4 + b]["part_out"]).T[::-1]
        out[b] = pf + pb + x[b]
    return out


def kernel(**inputs):
    inp = {k: np.asarray(v) for k, v in inputs.items()}
    in_maps = make_in_maps(inp)
    from concourse.bass_utils import run_bass_kernel_spmd
    nc = _get_nc()
    res = run_bass_kernel_spmd(nc, in_maps, core_ids=list(range(8)))
    return gather(inp["x"].astype(np.float32), res.results)
